# revision 1
# baseline (speedup 1.0000x reference)
"""Bayesian LSTM Trainium2 kernel (8 NeuronCores, data-parallel over batch).

Strategy:
  - Shard B=512 over 8 cores -> 64 batch rows/core -> M = 64*2 = 128 matmul rows.
  - Per step t: gates[128, 4*512] = comb[128,513] @ (Wmu + softplus(Wrho)*Weps_t) + bias_t
    computed as TWO accumulating matmul groups (static Wmu, streamed E=sig*eps)
    plus K=1 rank-one rows for the x-input column and the bias, all in fp32r.
  - Per-gate PSUM banks; ACT does sigmoid/tanh straight out of PSUM; DVE does
    the C/H elementwise updates; PE transposes H each step for the next lhsT.
  - Weps is streamed fp32 (host only re-lays-out, never changes values):
    one 4 MiB HWDGE DMA per step, double buffered.
"""

import os
import sys

import numpy as np

sys.path.insert(0, "/opt/trn_rl_repo")

import concourse.bass as bass  # noqa: E402
import concourse.tile as tile  # noqa: E402
from concourse import bacc, mybir  # noqa: E402
from concourse.bass_utils import run_bass_kernel_spmd  # noqa: E402
from concourse.masks import make_identity  # noqa: E402

B, T, H = 512, 128, 512
I = 1 + H
NCORES = 8
BS = B // NCORES          # 64 batch rows per core
M = BS * 2                # 128 matmul rows per core
GO = 4 * H                # 2048 gate outputs
NKT = 4                   # K-tiles over H (512 = 4*128)
F32 = mybir.dt.float32
F32R = mybir.dt.float32r
AF = mybir.ActivationFunctionType

LAST_EXEC_NS = None


def _r(ap):
    """bitcast an AP to float32r for full-rate PE matmuls."""
    return ap.bitcast(F32R)


def build_program(t_steps=T):
    nc = bacc.Bacc("TRN2", target_bir_lowering=False, debug=False)

    # ---- per-core DRAM I/O ----
    d_eps = nc.dram_tensor("eps_main", [t_steps, NKT, 128, GO], F32,
                           kind="ExternalInput").ap()   # Weps[t, 1+i, (g,o)] tiled by K
    d_eps0 = nc.dram_tensor("eps_row0", [t_steps, GO], F32, kind="ExternalInput").ap()
    d_wmu = nc.dram_tensor("wmu_main", [NKT, 128, GO], F32, kind="ExternalInput").ap()
    d_wrho = nc.dram_tensor("wrho_main", [NKT, 128, GO], F32, kind="ExternalInput").ap()
    d_wmu0 = nc.dram_tensor("wmu_row0", [1, GO], F32, kind="ExternalInput").ap()
    d_wrho0 = nc.dram_tensor("wrho_row0", [1, GO], F32, kind="ExternalInput").ap()
    d_beps = nc.dram_tensor("beps_r", [t_steps, GO], F32, kind="ExternalInput").ap()
    d_bmu = nc.dram_tensor("bmu_r", [1, GO], F32, kind="ExternalInput").ap()
    d_brho = nc.dram_tensor("brho_r", [1, GO], F32, kind="ExternalInput").ap()
    d_x = nc.dram_tensor("x_r", [t_steps, M], F32, kind="ExternalInput").ap()
    d_h0 = nc.dram_tensor("h0_r", [M, H], F32, kind="ExternalInput").ap()
    d_c0 = nc.dram_tensor("c0_r", [M, H], F32, kind="ExternalInput").ap()
    d_fw = nc.dram_tensor("fw_r", [128, NKT, 3], F32, kind="ExternalInput").ap()  # mu,rho,eps
    d_fb = nc.dram_tensor("fb_r", [1, 3], F32, kind="ExternalInput").ap()
    d_out = nc.dram_tensor("out_r", [M, 1], F32, kind="ExternalOutput").ap()

    with tile.TileContext(nc) as tc:
        _build_body(tc, t_steps, d_eps, d_eps0, d_wmu, d_wrho, d_wmu0, d_wrho0,
                    d_beps, d_bmu, d_brho, d_x, d_h0, d_c0, d_fw, d_fb, d_out)
    nc.compile()
    return nc


def _build_body(tc, t_steps, d_eps, d_eps0, d_wmu, d_wrho, d_wmu0, d_wrho0,
                d_beps, d_bmu, d_brho, d_x, d_h0, d_c0, d_fw, d_fb, d_out):
    nc = tc.nc

    def softplus_(ap):
        # softplus(x) = log(1 + exp(x)); Softplus has no ACT table set
        nc.scalar.activation(ap, ap, AF.Exp)
        nc.vector.tensor_scalar_add(ap, ap, 1.0)
        nc.scalar.activation(ap, ap, AF.Ln)

    from contextlib import ExitStack
    ctx = ExitStack()
    with ctx:
        statics = ctx.enter_context(tc.tile_pool(name="statics", bufs=1))
        epsp = ctx.enter_context(tc.tile_pool(name="eps", bufs=2))
        combp = ctx.enter_context(tc.tile_pool(name="comb", bufs=2))
        actp = ctx.enter_context(tc.tile_pool(name="acts", bufs=1))
        ebufp = ctx.enter_context(tc.tile_pool(name="ebuf", bufs=2))
        rows5 = ctx.enter_context(tc.tile_pool(name="rows5", bufs=2))
        gps = ctx.enter_context(tc.tile_pool(name="gpsum", bufs=1, space="PSUM"))
        trps = ctx.enter_context(tc.tile_pool(name="trpsum", bufs=2, space="PSUM"))
        bcps = ctx.enter_context(tc.tile_pool(name="bcpsum", bufs=2, space="PSUM"))

        # ---------------- static loads ----------------
        wmu = statics.tile([128, NKT, GO], F32R)
        for kt in range(NKT):
            stg = rows5.tile([128, GO], F32, tag="wbrow")
            nc.sync.dma_start(stg[:], d_wmu[kt])
            nc.vector.tensor_scalar_add(wmu[:, kt, :], stg[:], 0.0)
        sig = statics.tile([128, NKT, GO], F32)
        nc.sync.dma_start(sig[:], d_wrho.rearrange("k p n -> p k n"))
        softplus_(sig[:])   # sigma = softplus(rho)

        x_sb = statics.tile([t_steps, M], F32)
        nc.sync.dma_start(x_sb[:], d_x[:])
        x_sbr = statics.tile([t_steps, M], F32R)
        nc.vector.tensor_scalar_add(x_sbr[:], x_sb[:], 0.0)
        ones = statics.tile([1, M], F32)
        nc.vector.memset(ones[:], 1.0)
        ones_r = statics.tile([1, M], F32R)
        nc.vector.tensor_scalar_add(ones_r[:], ones[:], 0.0)
        ident = statics.tile([128, 128], F32)
        make_identity(nc, ident[:])

        # persistent state
        c_t = statics.tile([M, H], F32)
        nc.sync.dma_start(c_t[:], d_c0[:])
        h_sb = statics.tile([M, H], F32)
        nc.sync.dma_start(h_sb[:], d_h0[:])

        # ---------------- startup: broadcast rows, W0_full, B_all ----------------
        def load_row(src, softplus):
            row = rows5.tile([1, GO], F32, tag="wbrow")
            nc.sync.dma_start(row[:], src)
            if softplus:
                softplus_(row[:])
            return row

        def broadcast_row(row_ap):
            # rep[p, n] = row[0, n] via PE: ones^T @ row
            rep = rows5.tile([128, GO], F32, tag="wbrow")
            for ch in range(4):
                ps = bcps.tile([128, 512], F32, tag="bc")
                nc.tensor.matmul(ps[:], ones[:, 0:128], row_ap[:, ch * 512:(ch + 1) * 512],
                                 start=True, stop=True)
                nc.scalar.copy(rep[:, ch * 512:(ch + 1) * 512], ps[:])
            return rep

        w0_full = statics.tile([t_steps, GO], F32)   # Wmu0 + sig0*eps0[t]
        b_all = statics.tile([t_steps, GO], F32)     # bias_t rows

        nc.sync.dma_start(w0_full[:], d_eps0[:])
        rep = broadcast_row(load_row(d_wrho0[:], True)[:])
        nc.vector.tensor_mul(w0_full[:], w0_full[:], rep[0:t_steps, :])
        rep = broadcast_row(load_row(d_wmu0[:], False)[:])
        nc.vector.tensor_add(w0_full[:].bitcast(F32R), w0_full[:], rep[0:t_steps, :])

        nc.sync.dma_start(b_all[:], d_beps[:])
        rep = broadcast_row(load_row(d_brho[:], True)[:])
        nc.vector.tensor_mul(b_all[:], b_all[:], rep[0:t_steps, :])
        rep = broadcast_row(load_row(d_bmu[:], False)[:])
        nc.vector.tensor_add(b_all[:].bitcast(F32R), b_all[:], rep[0:t_steps, :])

        # ---------------- helpers ----------------
        def transpose_h(src_sb):
            """[128(bc), 512(h)] -> combT [128(h-chunk), kt, 128(bc)] in SBUF."""
            ps = trps.tile([128, NKT, 128], F32, tag="tr")
            for kt in range(NKT):
                nc.tensor.transpose(ps[:, kt, :], src_sb[:, kt * 128:(kt + 1) * 128],
                                    ident[:])
            comb = combp.tile([128, NKT, 128], F32R, tag="combT")
            nc.scalar.activation(comb[:], ps[:], AF.Copy)
            return comb

        comb = transpose_h(h_sb[:])

        # ---------------- the scan ----------------
        for t in range(t_steps):
            # stream this step's eps and make E = sig * eps in place
            eps = epsp.tile([128, NKT, GO], F32, tag="eps")
            nc.sync.dma_start(eps[:], d_eps[t].rearrange("k p n -> p k n"))

            # stage [x_t; ones] and [W0_full[t]; bias_t] at partition base 0
            xst = rows5.tile([1, M], F32R, tag="xst")
            nc.sync.dma_start(xst[:], x_sbr[t:t + 1, :])
            w0t = rows5.tile([1, GO], F32R, tag="wbrow")
            nc.sync.dma_start(w0t[:], w0_full[t:t + 1, :].bitcast(F32R))
            bt = rows5.tile([1, GO], F32R, tag="wbrow")
            nc.sync.dma_start(bt[:], b_all[t:t + 1, :].bitcast(F32R))

            gates = gps.tile([128, 4, 512], F32, tag="gates")
            # rank-1 rows first: comb-independent, so PE can run these while
            # the previous step's H-chain is still in flight (keeps HAM warm)
            for g in range(4):
                gsl = slice(g * 512, (g + 1) * 512)
                nc.tensor.matmul(gates[:, g, :], xst[:],
                                 w0t[:, gsl], start=True, stop=False)
                nc.tensor.matmul(gates[:, g, :], ones_r[:],
                                 bt[:, gsl], start=False, stop=False)
            for kt in range(NKT):
                ebuf = ebufp.tile([128, GO], F32R, tag="e")
                eng = nc.gpsimd if kt % 2 == 1 else nc.vector
                eng.tensor_mul(ebuf[:], eps[:, kt, :], sig[:, kt, :])
                for g in range(4):
                    gsl = slice(g * 512, (g + 1) * 512)
                    nc.tensor.matmul(gates[:, g, :], comb[:, kt, :],
                                     wmu[:, kt, gsl], start=False, stop=False)
                    nc.tensor.matmul(gates[:, g, :], comb[:, kt, :],
                                     ebuf[:, gsl], start=False,
                                     stop=(kt == NKT - 1))

            # activations straight out of PSUM (per-gate banks)
            i_sb = actp.tile([M, 512], F32, tag="i")
            nc.scalar.activation(i_sb[:], gates[:, 0, :], AF.Sigmoid)
            f_sb = actp.tile([M, 512], F32, tag="f")
            nc.scalar.activation(f_sb[:], gates[:, 1, :], AF.Sigmoid)
            ch_sb = actp.tile([M, 512], F32, tag="ch")
            nc.scalar.activation(ch_sb[:], gates[:, 2, :], AF.Tanh)
            o_sb = actp.tile([M, 512], F32, tag="o")
            nc.scalar.activation(o_sb[:], gates[:, 3, :], AF.Copy)

            # C_new = f*C + i*chat   (keep in persistent c_t)
            t2 = actp.tile([M, 512], F32, tag="t2")
            nc.vector.tensor_mul(t2[:], f_sb[:], c_t[:])
            t1 = actp.tile([M, 512], F32, tag="t1")
            nc.vector.tensor_mul(t1[:], i_sb[:], ch_sb[:])
            nc.vector.tensor_add(c_t[:], t1[:], t2[:])

            th = actp.tile([M, 512], F32, tag="th")
            nc.scalar.activation(th[:], c_t[:], AF.Tanh)
            h_new = actp.tile([M, 512], F32, tag="h")
            nc.vector.tensor_mul(h_new[:], o_sb[:], th[:])

            comb = transpose_h(h_new[:])

        # ---------------- final linear head ----------------
        fw = statics.tile([128, NKT, 3], F32)
        nc.sync.dma_start(fw[:], d_fw[:])
        fwt = statics.tile([128, NKT], F32)
        nc.vector.tensor_copy(fwt[:], fw[:, :, 1])
        softplus_(fwt[:])                                               # softplus(fWrho)
        nc.vector.tensor_mul(fwt[:], fwt[:], fw[:, :, 2])               # * fWeps
        fwv = statics.tile([128, NKT], F32R)
        nc.vector.tensor_add(fwv[:], fwt[:], fw[:, :, 0])               # + fWmu

        fb = statics.tile([1, 3], F32)
        nc.sync.dma_start(fb[:], d_fb[:])
        fbt = statics.tile([1, 1], F32)
        nc.vector.tensor_copy(fbt[:], fb[:, 1:2])
        softplus_(fbt[:])
        nc.vector.tensor_mul(fbt[:], fbt[:], fb[:, 2:3])
        fbv = statics.tile([1, 1], F32R)
        nc.vector.tensor_add(fbv[:], fbt[:], fb[:, 0:1])

        out_ps = bcps.tile([128, 512], F32, tag="bc")
        for kt in range(NKT):
            nc.tensor.matmul(out_ps[:, 0:1], comb[:, kt, :].bitcast(F32),
                             fwv[:, kt:kt + 1].bitcast(F32), start=(kt == 0), stop=False)
        nc.tensor.matmul(out_ps[:, 0:1], ones[:], fbv[:].bitcast(F32),
                         start=False, stop=True)
        out_sb = statics.tile([M, 1], F32)
        nc.vector.tensor_copy(out_sb[:], out_ps[:, 0:1])
        nc.sync.dma_start(d_out[:], out_sb[:])


_CACHE = {}


def _get_program(t_steps=T):
    if t_steps not in _CACHE:
        _CACHE[t_steps] = build_program(t_steps)
    return _CACHE[t_steps]


def prepare_inputs(x, H0, C0, Wmu, Wrho, Bmu, Brho, fWmu, fWrho, fBmu, fBrho,
                   Weps, Beps, fWeps, fBeps):
    """Host-side layout-only rearrangement (no value changes) + per-core maps."""
    t_steps = Weps.shape[0]
    # Weps [T,4,I,H] -> [T, I, 4*H]; split i=0 row from i>=1 body; body K-tiled.
    w_eps = np.ascontiguousarray(np.transpose(Weps, (0, 2, 1, 3))).reshape(t_steps, I, GO)
    eps_main = np.ascontiguousarray(w_eps[:, 1:, :]).reshape(t_steps, NKT, 128, GO)
    eps_row0 = np.ascontiguousarray(w_eps[:, 0, :])
    w_mu = np.ascontiguousarray(np.transpose(Wmu, (1, 0, 2))).reshape(I, GO)
    w_rho = np.ascontiguousarray(np.transpose(Wrho, (1, 0, 2))).reshape(I, GO)
    wmu_main = np.ascontiguousarray(w_mu[1:]).reshape(NKT, 128, GO)
    wrho_main = np.ascontiguousarray(w_rho[1:]).reshape(NKT, 128, GO)
    wmu_row0 = np.ascontiguousarray(w_mu[0:1])
    wrho_row0 = np.ascontiguousarray(w_rho[0:1])
    beps_r = np.ascontiguousarray(Beps.reshape(t_steps, GO))
    bmu_r = np.ascontiguousarray(Bmu.reshape(1, GO))
    brho_r = np.ascontiguousarray(Brho.reshape(1, GO))
    # fW* [H,1] -> [128, NKT] (h = kt*128 + p); stack mu/rho/eps
    def fw_lay(a):
        return np.ascontiguousarray(a.reshape(NKT, 128).T)
    fw_r = np.ascontiguousarray(np.stack([fw_lay(fWmu), fw_lay(fWrho), fw_lay(fWeps)], axis=-1))
    fb_r = np.ascontiguousarray(np.stack([fBmu.reshape(()), fBrho.reshape(()),
                                          fBeps.reshape(())]).reshape(1, 3))

    shared = {
        "eps_main": eps_main, "eps_row0": eps_row0,
        "wmu_main": wmu_main, "wrho_main": wrho_main,
        "wmu_row0": wmu_row0, "wrho_row0": wrho_row0,
        "beps_r": beps_r, "bmu_r": bmu_r, "brho_r": brho_r,
        "fw_r": fw_r, "fb_r": fb_r,
    }
    in_maps = []
    for c in range(NCORES):
        bsl = slice(c * BS, (c + 1) * BS)
        m = dict(shared)
        m["x_r"] = np.ascontiguousarray(np.transpose(x[bsl], (1, 0, 2)).reshape(t_steps, M))
        m["h0_r"] = np.ascontiguousarray(H0[bsl].reshape(M, H))
        m["c0_r"] = np.ascontiguousarray(C0[bsl].reshape(M, H))
        in_maps.append(m)
    return in_maps


def kernel(**inputs):
    global LAST_EXEC_NS
    t_steps = inputs["Weps"].shape[0]
    nc = _get_program(t_steps)
    in_maps = prepare_inputs(**inputs)
    trace = bool(int(os.environ.get("KERNEL_TRACE", "0")))
    res = run_bass_kernel_spmd(nc, in_maps, list(range(NCORES)), trace=trace)
    LAST_EXEC_NS = res.exec_time_ns
    out = np.empty((B, 2), dtype=np.float32)
    for c in range(NCORES):
        out[c * BS:(c + 1) * BS] = res.results[c]["out_r"].reshape(BS, 2)
    return out[:, None, :]

